# revision 36
# baseline (speedup 1.0000x reference)
"""SAGAN self-attention block on 8 TRN2 NeuronCores.

Reference (per batch element b, N = H*W = 4096, C = 512, D = 64):
    f = x @ Wf + bf ; g = x @ Wg + bg ; h = x @ Wh + bh      # [N, D]
    s = f @ g.T                                              # [N, N]
    attn = softmax(s, axis=-1)
    ctx = attn @ h                                           # [N, D]
    o = (gamma * ctx) @ Wv + bv + x                          # [N, C]

Sharding: data-parallel over batch B=8 -> one batch element per core, no
collectives. Weights replicated.

Device algorithm (per core), matmuls in bf16 with f32 PSUM accumulation:
  - load x [4096, 512] f32; transpose 128x128 blocks on the PE (f32), cast
    to bf16 on the PSUM->SBUF copy -> xT (c on partitions)
  - f and g projected in ONE matmul chain per 512-chunk using stacked
    [Wf|Wg] weights (M=128): fT lands on PSUM partitions 0:64, gT on
    64:128; each half is then mirrored into the other partition half of
    FT2/GT2 via SBUF->SBUF DMA so QK pairs can row-pack.
  - h_aug[m, :] = [x@Wh + bh, 1.0]  -> [4096, 65] bf16 (m on partitions);
    bh is applied by a K=1 matmul against a ones row.
  - unnormalized softmax (no max subtraction: |s| <~ 50 so exp fits f32/bf16):
      for each n-chunk of 512 columns:
        for each pair (i0, i1) of 128-row m-tiles:
          S'[i0]|S'[i1] computed CONCURRENTLY via K=64 row-packing
          (tile_position (0,0) and (64,0)) into one [128, 1024] PSUM tile
          E' = exp(S')  (ScalarE, one 1024-wide call)  -> SBUF bf16
          ctxT[0:65, chunk] += haug[i0].T @ E'[:, :512] + haug[i1].T @ E'[:, 512:]
        row 64 of ctxT = sum_m E' = softmax denominator (ones column trick)
  - out[n, :] = (ctxT[:, n].T @ [gamma*Wv ; bv]) * (1/denom[n]) + x[n, :]
      (bv rides on the denom row so it survives the 1/denom scaling)
"""

import numpy as np
import ml_dtypes

BF16 = ml_dtypes.bfloat16

B, HH, WW, C = 8, 64, 64, 512
D = C // 8          # 64
N_FULL = HH * WW    # 4096
P = 128
CC = C // P         # 4  (c-chunks of 128)

_CACHE: dict = {}


def _build(n: int):
    """Build + compile the single-core Bass program (same NEFF on all 8 cores)."""
    import concourse.mybir as mybir
    from concourse import bacc
    from concourse.tile import TileContext

    f32 = mybir.dt.float32
    bf16 = mybir.dt.bfloat16
    ADD = mybir.AluOpType.add
    MULT = mybir.AluOpType.mult
    EXP = mybir.ActivationFunctionType.Exp

    n_tiles = n // P
    n_pairs = n_tiles // 2
    nch = n // 512          # number of 512-wide n-chunks

    nc = bacc.Bacc("TRN2", target_bir_lowering=False, debug=False)

    x_d = nc.dram_tensor("x", [n, C], f32, kind="ExternalInput")
    wfg_d = nc.dram_tensor("wfg", [C, 2 * D], bf16, kind="ExternalInput")
    wh_d = nc.dram_tensor("wh", [C, D], bf16, kind="ExternalInput")
    bfg_d = nc.dram_tensor("bfg", [P, 1], f32, kind="ExternalInput")   # [bf;bg]
    bh_d = nc.dram_tensor("bhp", [1, D], bf16, kind="ExternalInput")
    on_d = nc.dram_tensor("onesp", [1, P], bf16, kind="ExternalInput")
    wv_d = nc.dram_tensor("wv", [D + 1, C], bf16, kind="ExternalInput")
    id_d = nc.dram_tensor("ident", [P, P], bf16, kind="ExternalInput")
    out_d = nc.dram_tensor("out", [n, C], f32, kind="ExternalOutput")

    x_t = x_d.rearrange("(i p) c -> i p c", p=P)
    o_t = out_d.rearrange("(i p) c -> i p c", p=P)

    with TileContext(nc) as tc:
        with (
            tc.tile_pool(name="const", bufs=1) as cpool,
            tc.tile_pool(name="big", bufs=1) as bigpool,
            tc.tile_pool(name="xb", bufs=3) as xpool,
            tc.tile_pool(name="ep", bufs=4) as epool,
            tc.tile_pool(name="ct", bufs=3) as ctpool,
            tc.tile_pool(name="os", bufs=4) as opool,
            tc.tile_pool(name="sm", bufs=4) as smpool,
            tc.tile_pool(name="psA", bufs=2, space="PSUM") as psA,
            tc.tile_pool(name="psB", bufs=2, space="PSUM") as psB,
            tc.tile_pool(name="psC", bufs=2, space="PSUM") as psC,
        ):
            # ---- replicated constants -> SBUF
            wfg_sb = cpool.tile([P, CC, 2 * D], bf16)
            nc.sync.dma_start(wfg_sb, wfg_d.rearrange("(cc p) d -> p cc d", p=P))
            wh_sb = cpool.tile([P, CC, D], bf16)
            nc.sync.dma_start(wh_sb, wh_d.rearrange("(cc p) d -> p cc d", p=P))
            bfg_sb = cpool.tile([P, 1], f32)
            nc.sync.dma_start(bfg_sb, bfg_d[:, :])
            bh_sb = cpool.tile([1, D], bf16)
            nc.sync.dma_start(bh_sb, bh_d[:, :])
            ones_sb = cpool.tile([1, P], bf16)
            nc.sync.dma_start(ones_sb, on_d[:, :])
            wv_sb = cpool.tile([D + 1, C], bf16)
            nc.sync.dma_start(wv_sb, wv_d[:, :])
            id_sb = cpool.tile([P, P], bf16)
            nc.sync.dma_start(id_sb, id_d[:, :])

            # ---- persistent SBUF tensors
            xres = bigpool.tile([P, n_tiles, C], f32)    # x rows (residual + cast src)
            xT = bigpool.tile([P, CC, n], bf16)          # x transposed (c on partitions)
            FT2 = bigpool.tile([P, n], bf16)             # f.T duplicated in both halves
            GT2 = bigpool.tile([P, n], bf16)             # g.T duplicated in both halves
            haug = bigpool.tile([P, n_tiles, D + 1], bf16)
            nc.gpsimd.memset(haug[:, :, D:D + 1], 1.0)

            # ---- prologue: load x, transpose (PE), project f/g/h.
            # Emission is interleaved per x-tile so scheduler priorities follow
            # the dataflow: tile i's transposes, then h(i), then the f/g chunk
            # as soon as its 4 tiles are in.
            def emit_fg(jc):
                sl = slice(jc * 512, (jc + 1) * 512)
                fg = psC.tile([P, 512], f32, tag="oc", name=f"fg{jc}")
                for cc in range(CC):
                    nc.tensor.matmul(
                        fg, lhsT=wfg_sb[:, cc, :], rhs=xT[:, cc, sl],
                        start=(cc == 0), stop=(cc == CC - 1),
                    )
                nc.vector.tensor_scalar(FT2[0:D, sl], fg[0:D, :], bfg_sb[0:D], None, ADD)
                nc.vector.tensor_scalar(GT2[D:P, sl], fg[D:P, :], bfg_sb[D:P], None, ADD)
                nc.sync.dma_start(FT2[D:P, sl], FT2[0:D, sl])
                nc.sync.dma_start(GT2[0:D, sl], GT2[D:P, sl])

            def emit_h(i):
                hps = psC.tile([P, D], f32, tag="oc", name=f"hps{i}")
                for cc in range(CC):
                    nc.tensor.matmul(
                        hps, lhsT=xT[:, cc, i * P:(i + 1) * P], rhs=wh_sb[:, cc, :],
                        start=(cc == 0), stop=False,
                    )
                nc.tensor.matmul(hps, lhsT=ones_sb, rhs=bh_sb, start=False, stop=True)
                nc.vector.tensor_copy(out=haug[:, i, 0:D], in_=hps)

            H2 = P // 2
            for i in range(n_tiles):
                nc.sync.dma_start(xres[:, i, :], x_t[i])
                xb = xpool.tile([P, C], bf16, tag="xb")
                nc.vector.tensor_copy(out=xb, in_=xres[:, i, :])
                tp = psA.tile([P, C], f32, tag="sp")
                for cc in range(CC):
                    # transpose = plain matmul against the identity; two
                    # 64-column halves run concurrently in disjoint PE column
                    # groups and land on disjoint PSUM partition halves,
                    # rebuilding the full [128,128] transposed block.
                    base = cc * P
                    nc.tensor.matmul(
                        tp[0:H2, base:base + P], lhsT=xb[:, base:base + H2],
                        rhs=id_sb, start=True, stop=True, tile_position=(0, 0),
                    )
                    nc.tensor.matmul(
                        tp[H2:P, base:base + P], lhsT=xb[:, base + H2:base + P],
                        rhs=id_sb, start=True, stop=True, tile_position=(0, H2),
                    )
                nc.vector.tensor_copy(
                    out=xT[:, :, i * P:(i + 1) * P],
                    in_=tp.rearrange("p (cc q) -> p cc q", q=P),
                )
                emit_h(i)
                if i % 4 == 3:
                    emit_fg(i // 4)

            # ---- attention main loop: n-chunks of 512, m-tiles in packed pairs
            for jc in range(nch):
                sl = slice(jc * 512, (jc + 1) * 512)
                ctx = psB.tile([D + 1, 512], f32, tag="ctx")
                for ip in range(n_pairs):
                    i0, i1 = 2 * ip, 2 * ip + 1
                    sp = psA.tile([P, 1024], f32, tag="sp")
                    # two K=64 QK matmuls run concurrently in array row groups
                    nc.tensor.matmul(
                        sp[:, 0:512],
                        lhsT=GT2[0:D, i0 * P:(i0 + 1) * P], rhs=FT2[0:D, sl],
                        start=True, stop=True, tile_position=(0, 0),
                    )
                    nc.tensor.matmul(
                        sp[:, 512:1024],
                        lhsT=GT2[D:P, i1 * P:(i1 + 1) * P], rhs=FT2[D:P, sl],
                        start=True, stop=True, tile_position=(D, 0),
                    )
                    ep = epool.tile([P, 1024], bf16, tag="ep")
                    nc.scalar.activation(ep, sp, EXP)
                    nc.tensor.matmul(
                        ctx, lhsT=haug[:, i0, :], rhs=ep[:, 0:512],
                        start=(ip == 0), stop=False,
                    )
                    nc.tensor.matmul(
                        ctx, lhsT=haug[:, i1, :], rhs=ep[:, 512:1024],
                        start=False, stop=(ip == n_pairs - 1),
                    )

                # ---- epilogue for this n-chunk (4 subtiles of 128 rows)
                ct = ctpool.tile([D + 1, 512], bf16, tag="ct")
                nc.vector.tensor_copy(out=ct, in_=ctx)
                for t in range(4):
                    it = jc * 4 + t
                    tsl = slice(t * P, (t + 1) * P)
                    dt = psC.tile([P, 1], bf16, tag="oc")
                    nc.tensor.transpose(dt, ct[D:D + 1, tsl], id_sb[D:D + 1, D:D + 1])
                    rc = smpool.tile([P, 1], f32, tag="rc")
                    nc.vector.reciprocal(rc, dt)
                    op = psC.tile([P, C], f32, tag="oc")
                    nc.tensor.matmul(op, lhsT=ct[:, tsl], rhs=wv_sb, start=True, stop=True)
                    osb = opool.tile([P, C], f32, tag="os")
                    nc.vector.tensor_scalar(osb, op, rc, None, MULT)
                    nc.vector.tensor_tensor(osb, osb, xres[:, it, :], ADD)
                    nc.sync.dma_start(o_t[it], osb)

    nc.compile()
    return nc


def get_program(n: int = N_FULL):
    if n not in _CACHE:
        _CACHE[n] = _build(n)
    return _CACHE[n]


def make_weight_maps(Wf, bf, Wg, bg, Wh, bh, Wv, bv, gamma):
    """Host-side layout prep of the tiny replicated weights."""
    wv_aug = np.concatenate(
        [np.float32(gamma) * np.asarray(Wv, np.float32),
         np.asarray(bv, np.float32)[None, :]], axis=0)
    bfg = np.concatenate(
        [np.asarray(bf, np.float32), np.asarray(bg, np.float32)]).reshape(P, 1)
    wfg = np.concatenate(
        [np.asarray(Wf, np.float32), np.asarray(Wg, np.float32)], axis=1)
    return {
        "wfg": np.ascontiguousarray(wfg.astype(BF16)),
        "wh": np.ascontiguousarray(np.asarray(Wh, np.float32).astype(BF16)),
        "bfg": np.ascontiguousarray(bfg),
        "bhp": np.ascontiguousarray(np.asarray(bh, np.float32).astype(BF16).reshape(1, D)),
        "onesp": np.ones((1, P), dtype=BF16),
        "wv": np.ascontiguousarray(wv_aug.astype(BF16)),
        "ident": np.ascontiguousarray(np.eye(P, dtype=BF16)),
    }


def kernel(x, Wf, bf, Wg, bg, Wh, bh, Wv, bv, gamma):
    from concourse.bass_utils import run_bass_kernel_spmd

    x = np.asarray(x, np.float32)
    b, hh, ww, c = x.shape
    n = hh * ww
    assert (b, c) == (B, C)

    nc = get_program(n)
    base = make_weight_maps(Wf, bf, Wg, bg, Wh, bh, Wv, bv, gamma)
    xf = x.reshape(b, n, c)
    in_maps = [dict(base, x=np.ascontiguousarray(xf[i])) for i in range(b)]

    res = run_bass_kernel_spmd(nc, in_maps, core_ids=list(range(b)))
    out = np.stack([res.results[i]["out"] for i in range(b)], axis=0)
    return np.ascontiguousarray(out.reshape(b, hh, ww, c).astype(np.float32))


# revision 37
# speedup vs baseline: 1.0432x; 1.0432x over previous
"""SAGAN self-attention block on 8 TRN2 NeuronCores.

Reference (per batch element b, N = H*W = 4096, C = 512, D = 64):
    f = x @ Wf + bf ; g = x @ Wg + bg ; h = x @ Wh + bh      # [N, D]
    s = f @ g.T                                              # [N, N]
    attn = softmax(s, axis=-1)
    ctx = attn @ h                                           # [N, D]
    o = (gamma * ctx) @ Wv + bv + x                          # [N, C]

Sharding: data-parallel over batch B=8 -> one batch element per core, no
collectives. Weights replicated.

Device algorithm (per core), matmuls in bf16 with f32 PSUM accumulation:
  - load x [4096, 512] f32; transpose 128x128 blocks on the PE (f32), cast
    to bf16 on the PSUM->SBUF copy -> xT (c on partitions)
  - f and g projected in ONE matmul chain per 512-chunk using stacked
    [Wf|Wg] weights (M=128): fT lands on PSUM partitions 0:64, gT on
    64:128; each half is then mirrored into the other partition half of
    FT2/GT2 via SBUF->SBUF DMA so QK pairs can row-pack.
  - h_aug[m, :] = [x@Wh + bh, 1.0]  -> [4096, 65] bf16 (m on partitions);
    bh is applied by a K=1 matmul against a ones row.
  - unnormalized softmax (no max subtraction: |s| <~ 50 so exp fits f32/bf16):
      for each n-chunk of 512 columns:
        for each pair (i0, i1) of 128-row m-tiles:
          S'[i0]|S'[i1] computed CONCURRENTLY via K=64 row-packing
          (tile_position (0,0) and (64,0)) into one [128, 1024] PSUM tile
          E' = exp(S')  (ScalarE, one 1024-wide call)  -> SBUF bf16
          ctxT[0:65, chunk] += haug[i0].T @ E'[:, :512] + haug[i1].T @ E'[:, 512:]
        row 64 of ctxT = sum_m E' = softmax denominator (ones column trick)
  - out[n, :] = (ctxT[:, n].T @ [gamma*Wv ; bv]) * (1/denom[n]) + x[n, :]
      (bv rides on the denom row so it survives the 1/denom scaling)
"""

import numpy as np
import ml_dtypes

BF16 = ml_dtypes.bfloat16

B, HH, WW, C = 8, 64, 64, 512
D = C // 8          # 64
N_FULL = HH * WW    # 4096
P = 128
CC = C // P         # 4  (c-chunks of 128)

_CACHE: dict = {}


def _build(n: int):
    """Build + compile the single-core Bass program (same NEFF on all 8 cores)."""
    import concourse.mybir as mybir
    from concourse import bacc
    from concourse.tile import TileContext

    f32 = mybir.dt.float32
    bf16 = mybir.dt.bfloat16
    ADD = mybir.AluOpType.add
    MULT = mybir.AluOpType.mult
    EXP = mybir.ActivationFunctionType.Exp

    n_tiles = n // P
    n_pairs = n_tiles // 2
    nch = n // 512          # number of 512-wide n-chunks

    nc = bacc.Bacc("TRN2", target_bir_lowering=False, debug=False)

    x_d = nc.dram_tensor("x", [n, C], f32, kind="ExternalInput")
    wfg_d = nc.dram_tensor("wfg", [C, 2 * D], bf16, kind="ExternalInput")
    wh_d = nc.dram_tensor("wh", [C, D], bf16, kind="ExternalInput")
    bfg_d = nc.dram_tensor("bfg", [P, 1], f32, kind="ExternalInput")   # [bf;bg]
    bh_d = nc.dram_tensor("bhp", [1, D], bf16, kind="ExternalInput")
    on_d = nc.dram_tensor("onesp", [1, P], bf16, kind="ExternalInput")
    wv_d = nc.dram_tensor("wv", [D + 1, C], bf16, kind="ExternalInput")
    id_d = nc.dram_tensor("ident", [P, P], bf16, kind="ExternalInput")
    out_d = nc.dram_tensor("out", [n, C], f32, kind="ExternalOutput")

    x_t = x_d.rearrange("(i p) c -> i p c", p=P)
    o_t = out_d.rearrange("(i p) c -> i p c", p=P)

    with TileContext(nc) as tc:
        with (
            tc.tile_pool(name="const", bufs=1) as cpool,
            tc.tile_pool(name="big", bufs=1) as bigpool,
            tc.tile_pool(name="ep", bufs=4) as epool,
            tc.tile_pool(name="ct", bufs=3) as ctpool,
            tc.tile_pool(name="os", bufs=4) as opool,
            tc.tile_pool(name="sm", bufs=4) as smpool,
            tc.tile_pool(name="psA", bufs=2, space="PSUM") as psA,
            tc.tile_pool(name="psB", bufs=2, space="PSUM") as psB,
            tc.tile_pool(name="psC", bufs=2, space="PSUM") as psC,
        ):
            # ---- replicated constants -> SBUF
            wfg_sb = cpool.tile([P, CC, 2 * D], bf16)
            nc.sync.dma_start(wfg_sb, wfg_d.rearrange("(cc p) d -> p cc d", p=P))
            wh_sb = cpool.tile([P, CC, D], bf16)
            nc.sync.dma_start(wh_sb, wh_d.rearrange("(cc p) d -> p cc d", p=P))
            bfg_sb = cpool.tile([P, 1], f32)
            nc.sync.dma_start(bfg_sb, bfg_d[:, :])
            bh_sb = cpool.tile([1, D], bf16)
            nc.sync.dma_start(bh_sb, bh_d[:, :])
            ones_sb = cpool.tile([1, P], bf16)
            nc.sync.dma_start(ones_sb, on_d[:, :])
            wv_sb = cpool.tile([D + 1, C], bf16)
            nc.sync.dma_start(wv_sb, wv_d[:, :])
            id_sb = cpool.tile([P, P], bf16)
            nc.sync.dma_start(id_sb, id_d[:, :])
            idf_sb = cpool.tile([P, P], f32)
            nc.vector.tensor_copy(out=idf_sb, in_=id_sb)

            # ---- persistent SBUF tensors
            xres = bigpool.tile([P, n_tiles, C], f32)    # x rows (residual + cast src)
            xT = bigpool.tile([P, CC, n], bf16)          # x transposed (c on partitions)
            FT2 = bigpool.tile([P, n], bf16)             # f.T duplicated in both halves
            GT2 = bigpool.tile([P, n], bf16)             # g.T duplicated in both halves
            haug = bigpool.tile([P, n_tiles, D + 1], bf16)
            nc.gpsimd.memset(haug[:, :, D:D + 1], 1.0)

            # ---- prologue: load x, transpose (PE), project f/g/h.
            # Emission is interleaved per x-tile so scheduler priorities follow
            # the dataflow: tile i's transposes, then h(i), then the f/g chunk
            # as soon as its 4 tiles are in.
            def emit_fg(jc):
                sl = slice(jc * 512, (jc + 1) * 512)
                fg = psC.tile([P, 512], f32, tag="oc", name=f"fg{jc}")
                for cc in range(CC):
                    nc.tensor.matmul(
                        fg, lhsT=wfg_sb[:, cc, :], rhs=xT[:, cc, sl],
                        start=(cc == 0), stop=(cc == CC - 1),
                    )
                nc.vector.tensor_scalar(FT2[0:D, sl], fg[0:D, :], bfg_sb[0:D], None, ADD)
                nc.vector.tensor_scalar(GT2[D:P, sl], fg[D:P, :], bfg_sb[D:P], None, ADD)
                nc.sync.dma_start(FT2[D:P, sl], FT2[0:D, sl])
                nc.sync.dma_start(GT2[0:D, sl], GT2[D:P, sl])

            def emit_h(i):
                hps = psC.tile([P, D], f32, tag="oc", name=f"hps{i}")
                for cc in range(CC):
                    nc.tensor.matmul(
                        hps, lhsT=xT[:, cc, i * P:(i + 1) * P], rhs=wh_sb[:, cc, :],
                        start=(cc == 0), stop=False,
                    )
                nc.tensor.matmul(hps, lhsT=ones_sb, rhs=bh_sb, start=False, stop=True)
                nc.vector.tensor_copy(out=haug[:, i, 0:D], in_=hps)

            for i in range(n_tiles):
                nc.sync.dma_start(xres[:, i, :], x_t[i])
                tp = psA.tile([P, C], f32, tag="sp")
                for cc in range(CC):
                    nc.tensor.transpose(
                        tp[:, cc * P:(cc + 1) * P],
                        xres[:, i, cc * P:(cc + 1) * P], idf_sb
                    )
                nc.vector.tensor_copy(
                    out=xT[:, :, i * P:(i + 1) * P],
                    in_=tp.rearrange("p (cc q) -> p cc q", q=P),
                )
                emit_h(i)
                if i % 4 == 3:
                    emit_fg(i // 4)

            # ---- attention main loop: n-chunks of 512, m-tiles in packed pairs
            for jc in range(nch):
                sl = slice(jc * 512, (jc + 1) * 512)
                ctx = psB.tile([D + 1, 512], f32, tag="ctx")
                for ip in range(n_pairs):
                    i0, i1 = 2 * ip, 2 * ip + 1
                    sp = psA.tile([P, 1024], f32, tag="sp")
                    # two K=64 QK matmuls run concurrently in array row groups
                    nc.tensor.matmul(
                        sp[:, 0:512],
                        lhsT=GT2[0:D, i0 * P:(i0 + 1) * P], rhs=FT2[0:D, sl],
                        start=True, stop=True, tile_position=(0, 0),
                    )
                    nc.tensor.matmul(
                        sp[:, 512:1024],
                        lhsT=GT2[D:P, i1 * P:(i1 + 1) * P], rhs=FT2[D:P, sl],
                        start=True, stop=True, tile_position=(D, 0),
                    )
                    ep = epool.tile([P, 1024], bf16, tag="ep")
                    nc.scalar.activation(ep, sp, EXP)
                    nc.tensor.matmul(
                        ctx, lhsT=haug[:, i0, :], rhs=ep[:, 0:512],
                        start=(ip == 0), stop=False,
                    )
                    nc.tensor.matmul(
                        ctx, lhsT=haug[:, i1, :], rhs=ep[:, 512:1024],
                        start=False, stop=(ip == n_pairs - 1),
                    )

                # ---- epilogue for this n-chunk (4 subtiles of 128 rows)
                ct = ctpool.tile([D + 1, 512], bf16, tag="ct")
                nc.vector.tensor_copy(out=ct, in_=ctx)
                for t in range(4):
                    it = jc * 4 + t
                    tsl = slice(t * P, (t + 1) * P)
                    dt = psC.tile([P, 1], bf16, tag="oc")
                    nc.tensor.transpose(dt, ct[D:D + 1, tsl], id_sb[D:D + 1, D:D + 1])
                    rc = smpool.tile([P, 1], f32, tag="rc")
                    nc.vector.reciprocal(rc, dt)
                    op = psC.tile([P, C], f32, tag="oc")
                    nc.tensor.matmul(op, lhsT=ct[:, tsl], rhs=wv_sb, start=True, stop=True)
                    osb = opool.tile([P, C], f32, tag="os")
                    nc.vector.tensor_scalar(osb, op, rc, None, MULT)
                    nc.vector.tensor_tensor(osb, osb, xres[:, it, :], ADD)
                    nc.sync.dma_start(o_t[it], osb)

    nc.compile()
    return nc


def get_program(n: int = N_FULL):
    if n not in _CACHE:
        _CACHE[n] = _build(n)
    return _CACHE[n]


def make_weight_maps(Wf, bf, Wg, bg, Wh, bh, Wv, bv, gamma):
    """Host-side layout prep of the tiny replicated weights."""
    wv_aug = np.concatenate(
        [np.float32(gamma) * np.asarray(Wv, np.float32),
         np.asarray(bv, np.float32)[None, :]], axis=0)
    bfg = np.concatenate(
        [np.asarray(bf, np.float32), np.asarray(bg, np.float32)]).reshape(P, 1)
    wfg = np.concatenate(
        [np.asarray(Wf, np.float32), np.asarray(Wg, np.float32)], axis=1)
    return {
        "wfg": np.ascontiguousarray(wfg.astype(BF16)),
        "wh": np.ascontiguousarray(np.asarray(Wh, np.float32).astype(BF16)),
        "bfg": np.ascontiguousarray(bfg),
        "bhp": np.ascontiguousarray(np.asarray(bh, np.float32).astype(BF16).reshape(1, D)),
        "onesp": np.ones((1, P), dtype=BF16),
        "wv": np.ascontiguousarray(wv_aug.astype(BF16)),
        "ident": np.ascontiguousarray(np.eye(P, dtype=BF16)),
    }


def kernel(x, Wf, bf, Wg, bg, Wh, bh, Wv, bv, gamma):
    from concourse.bass_utils import run_bass_kernel_spmd

    x = np.asarray(x, np.float32)
    b, hh, ww, c = x.shape
    n = hh * ww
    assert (b, c) == (B, C)

    nc = get_program(n)
    base = make_weight_maps(Wf, bf, Wg, bg, Wh, bh, Wv, bv, gamma)
    xf = x.reshape(b, n, c)
    in_maps = [dict(base, x=np.ascontiguousarray(xf[i])) for i in range(b)]

    res = run_bass_kernel_spmd(nc, in_maps, core_ids=list(range(b)))
    out = np.stack([res.results[i]["out"] for i in range(b)], axis=0)
    return np.ascontiguousarray(out.reshape(b, hh, ww, c).astype(np.float32))


# revision 38
# speedup vs baseline: 1.0730x; 1.0285x over previous
"""SAGAN self-attention block on 8 TRN2 NeuronCores.

Reference (per batch element b, N = H*W = 4096, C = 512, D = 64):
    f = x @ Wf + bf ; g = x @ Wg + bg ; h = x @ Wh + bh      # [N, D]
    s = f @ g.T                                              # [N, N]
    attn = softmax(s, axis=-1)
    ctx = attn @ h                                           # [N, D]
    o = (gamma * ctx) @ Wv + bv + x                          # [N, C]

Sharding: data-parallel over batch B=8 -> one batch element per core, no
collectives. Weights replicated.

Device algorithm (per core), matmuls in bf16 with f32 PSUM accumulation:
  - load x [4096, 512] f32; transpose 128x128 blocks on the PE (f32), cast
    to bf16 on the PSUM->SBUF copy -> xT (c on partitions)
  - f and g projected in ONE matmul chain per 512-chunk using stacked
    [Wf|Wg] weights (M=128): fT lands on PSUM partitions 0:64, gT on
    64:128; each half is then mirrored into the other partition half of
    FT2/GT2 via SBUF->SBUF DMA so QK pairs can row-pack.
  - h_aug[m, :] = [x@Wh + bh, 1.0]  -> [4096, 65] bf16 (m on partitions);
    bh is applied by a K=1 matmul against a ones row.
  - unnormalized softmax (no max subtraction: |s| <~ 50 so exp fits f32/bf16):
      for each n-chunk of 512 columns:
        for each pair (i0, i1) of 128-row m-tiles:
          S'[i0]|S'[i1] computed CONCURRENTLY via K=64 row-packing
          (tile_position (0,0) and (64,0)) into one [128, 1024] PSUM tile
          E' = exp(S')  (ScalarE, one 1024-wide call)  -> SBUF bf16
          ctxT[0:65, chunk] += haug[i0].T @ E'[:, :512] + haug[i1].T @ E'[:, 512:]
        row 64 of ctxT = sum_m E' = softmax denominator (ones column trick)
  - out[n, :] = (ctxT[:, n].T @ [gamma*Wv ; bv]) * (1/denom[n]) + x[n, :]
      (bv rides on the denom row so it survives the 1/denom scaling)
"""

import numpy as np
import ml_dtypes

BF16 = ml_dtypes.bfloat16

B, HH, WW, C = 8, 64, 64, 512
D = C // 8          # 64
N_FULL = HH * WW    # 4096
P = 128
CC = C // P         # 4  (c-chunks of 128)

_CACHE: dict = {}


def _build(n: int, h_bias_zero: bool = False):
    """Build + compile the single-core Bass program (same NEFF on all 8 cores)."""
    import concourse.mybir as mybir
    from concourse import bacc
    from concourse.tile import TileContext

    f32 = mybir.dt.float32
    bf16 = mybir.dt.bfloat16
    ADD = mybir.AluOpType.add
    MULT = mybir.AluOpType.mult
    EXP = mybir.ActivationFunctionType.Exp

    n_tiles = n // P
    n_pairs = n_tiles // 2
    nch = n // 512          # number of 512-wide n-chunks

    nc = bacc.Bacc("TRN2", target_bir_lowering=False, debug=False)

    x_d = nc.dram_tensor("x", [n, C], f32, kind="ExternalInput")
    wfg_d = nc.dram_tensor("wfg", [C, 2 * D], bf16, kind="ExternalInput")
    wh_d = nc.dram_tensor("wh", [C, D], bf16, kind="ExternalInput")
    bfg_d = nc.dram_tensor("bfg", [P, 1], f32, kind="ExternalInput")   # [bf;bg]
    if not h_bias_zero:
        bh_d = nc.dram_tensor("bhp", [1, D], bf16, kind="ExternalInput")
        on_d = nc.dram_tensor("onesp", [1, P], bf16, kind="ExternalInput")
    wv_d = nc.dram_tensor("wv", [D + 1, C], bf16, kind="ExternalInput")
    id_d = nc.dram_tensor("ident", [P, P], bf16, kind="ExternalInput")
    out_d = nc.dram_tensor("out", [n, C], f32, kind="ExternalOutput")

    x_t = x_d.rearrange("(i p) c -> i p c", p=P)
    o_t = out_d.rearrange("(i p) c -> i p c", p=P)

    with TileContext(nc) as tc:
        with (
            tc.tile_pool(name="const", bufs=1) as cpool,
            tc.tile_pool(name="big", bufs=1) as bigpool,
            tc.tile_pool(name="ep", bufs=4) as epool,
            tc.tile_pool(name="ct", bufs=3) as ctpool,
            tc.tile_pool(name="os", bufs=4) as opool,
            tc.tile_pool(name="sm", bufs=4) as smpool,
            tc.tile_pool(name="psA", bufs=2, space="PSUM") as psA,
            tc.tile_pool(name="psB", bufs=2, space="PSUM") as psB,
            tc.tile_pool(name="psC", bufs=2, space="PSUM") as psC,
        ):
            # ---- replicated constants -> SBUF
            wfg_sb = cpool.tile([P, CC, 2 * D], bf16)
            nc.sync.dma_start(wfg_sb, wfg_d.rearrange("(cc p) d -> p cc d", p=P))
            wh_sb = cpool.tile([P, CC, D], bf16)
            nc.sync.dma_start(wh_sb, wh_d.rearrange("(cc p) d -> p cc d", p=P))
            bfg_sb = cpool.tile([P, 1], f32)
            nc.sync.dma_start(bfg_sb, bfg_d[:, :])
            if not h_bias_zero:
                bh_sb = cpool.tile([1, D], bf16)
                nc.sync.dma_start(bh_sb, bh_d[:, :])
                ones_sb = cpool.tile([1, P], bf16)
                nc.sync.dma_start(ones_sb, on_d[:, :])
            wv_sb = cpool.tile([D + 1, C], bf16)
            nc.sync.dma_start(wv_sb, wv_d[:, :])
            id_sb = cpool.tile([P, P], bf16)
            nc.sync.dma_start(id_sb, id_d[:, :])
            idf_sb = cpool.tile([P, P], f32)
            nc.vector.tensor_copy(out=idf_sb, in_=id_sb)

            # ---- persistent SBUF tensors
            xres = bigpool.tile([P, n_tiles, C], f32)    # x rows (residual + cast src)
            xT = bigpool.tile([P, CC, n], bf16)          # x transposed (c on partitions)
            FT2 = bigpool.tile([P, n], bf16)             # f.T duplicated in both halves
            GT2 = bigpool.tile([P, n], bf16)             # g.T duplicated in both halves
            haug = bigpool.tile([P, n_tiles, D + 1], bf16)
            nc.gpsimd.memset(haug[:, :, D:D + 1], 1.0)

            # ---- prologue: load x, transpose (PE), project f/g/h.
            # Emission is interleaved per x-tile so scheduler priorities follow
            # the dataflow: tile i's transposes, then h(i), then the f/g chunk
            # as soon as its 4 tiles are in.
            def emit_fg(jc):
                sl = slice(jc * 512, (jc + 1) * 512)
                fg = psC.tile([P, 512], f32, tag="oc", name=f"fg{jc}")
                for cc in range(CC):
                    nc.tensor.matmul(
                        fg, lhsT=wfg_sb[:, cc, :], rhs=xT[:, cc, sl],
                        start=(cc == 0), stop=(cc == CC - 1),
                    )
                nc.vector.tensor_scalar(FT2[0:D, sl], fg[0:D, :], bfg_sb[0:D], None, ADD)
                nc.vector.tensor_scalar(GT2[D:P, sl], fg[D:P, :], bfg_sb[D:P], None, ADD)
                nc.sync.dma_start(FT2[D:P, sl], FT2[0:D, sl])
                nc.sync.dma_start(GT2[0:D, sl], GT2[D:P, sl])

            def emit_h(i):
                hps = psC.tile([P, D], f32, tag="oc", name=f"hps{i}")
                for cc in range(CC):
                    nc.tensor.matmul(
                        hps, lhsT=xT[:, cc, i * P:(i + 1) * P], rhs=wh_sb[:, cc, :],
                        start=(cc == 0), stop=(h_bias_zero and cc == CC - 1),
                    )
                if not h_bias_zero:
                    nc.tensor.matmul(
                        hps, lhsT=ones_sb, rhs=bh_sb, start=False, stop=True)
                nc.vector.tensor_copy(out=haug[:, i, 0:D], in_=hps)

            for i in range(n_tiles):
                nc.sync.dma_start(xres[:, i, :], x_t[i])
                tp = psA.tile([P, C], f32, tag="sp")
                for cc in range(CC):
                    nc.tensor.transpose(
                        tp[:, cc * P:(cc + 1) * P],
                        xres[:, i, cc * P:(cc + 1) * P], idf_sb
                    )
                nc.vector.tensor_copy(
                    out=xT[:, :, i * P:(i + 1) * P],
                    in_=tp.rearrange("p (cc q) -> p cc q", q=P),
                )
                emit_h(i)
                if i % 4 == 3:
                    emit_fg(i // 4)

            # ---- attention main loop: n-chunks of 512, m-tiles in packed pairs
            for jc in range(nch):
                sl = slice(jc * 512, (jc + 1) * 512)
                ctx = psB.tile([D + 1, 512], f32, tag="ctx")
                for ip in range(n_pairs):
                    i0, i1 = 2 * ip, 2 * ip + 1
                    sp = psA.tile([P, 1024], f32, tag="sp")
                    # two K=64 QK matmuls run concurrently in array row groups
                    nc.tensor.matmul(
                        sp[:, 0:512],
                        lhsT=GT2[0:D, i0 * P:(i0 + 1) * P], rhs=FT2[0:D, sl],
                        start=True, stop=True, tile_position=(0, 0),
                    )
                    nc.tensor.matmul(
                        sp[:, 512:1024],
                        lhsT=GT2[D:P, i1 * P:(i1 + 1) * P], rhs=FT2[D:P, sl],
                        start=True, stop=True, tile_position=(D, 0),
                    )
                    ep = epool.tile([P, 1024], bf16, tag="ep")
                    nc.scalar.activation(ep, sp, EXP)
                    nc.tensor.matmul(
                        ctx, lhsT=haug[:, i0, :], rhs=ep[:, 0:512],
                        start=(ip == 0), stop=False,
                    )
                    nc.tensor.matmul(
                        ctx, lhsT=haug[:, i1, :], rhs=ep[:, 512:1024],
                        start=False, stop=(ip == n_pairs - 1),
                    )

                # ---- epilogue for this n-chunk (4 subtiles of 128 rows)
                ct = ctpool.tile([D + 1, 512], bf16, tag="ct")
                nc.vector.tensor_copy(out=ct, in_=ctx)
                for t in range(4):
                    it = jc * 4 + t
                    tsl = slice(t * P, (t + 1) * P)
                    dt = psC.tile([P, 1], bf16, tag="oc")
                    nc.tensor.transpose(dt, ct[D:D + 1, tsl], id_sb[D:D + 1, D:D + 1])
                    rc = smpool.tile([P, 1], f32, tag="rc")
                    nc.vector.reciprocal(rc, dt)
                    op = psC.tile([P, C], f32, tag="oc")
                    nc.tensor.matmul(op, lhsT=ct[:, tsl], rhs=wv_sb, start=True, stop=True)
                    osb = opool.tile([P, C], f32, tag="os")
                    nc.vector.tensor_scalar(osb, op, rc, None, MULT)
                    nc.vector.tensor_tensor(osb, osb, xres[:, it, :], ADD)
                    nc.sync.dma_start(o_t[it], osb)

    nc.compile()
    return nc


def get_program(n: int = N_FULL, h_bias_zero: bool = False):
    key = (n, h_bias_zero)
    if key not in _CACHE:
        _CACHE[key] = _build(n, h_bias_zero)
    return _CACHE[key]


def make_weight_maps(Wf, bf, Wg, bg, Wh, bh, Wv, bv, gamma, h_bias_zero=False):
    """Host-side layout prep of the tiny replicated weights."""
    wv_aug = np.concatenate(
        [np.float32(gamma) * np.asarray(Wv, np.float32),
         np.asarray(bv, np.float32)[None, :]], axis=0)
    bfg = np.concatenate(
        [np.asarray(bf, np.float32), np.asarray(bg, np.float32)]).reshape(P, 1)
    wfg = np.concatenate(
        [np.asarray(Wf, np.float32), np.asarray(Wg, np.float32)], axis=1)
    maps = {
        "wfg": np.ascontiguousarray(wfg.astype(BF16)),
        "wh": np.ascontiguousarray(np.asarray(Wh, np.float32).astype(BF16)),
        "bfg": np.ascontiguousarray(bfg),
        "bhp": np.ascontiguousarray(np.asarray(bh, np.float32).astype(BF16).reshape(1, D)),
        "onesp": np.ones((1, P), dtype=BF16),
        "wv": np.ascontiguousarray(wv_aug.astype(BF16)),
        "ident": np.ascontiguousarray(np.eye(P, dtype=BF16)),
    }
    if h_bias_zero:
        del maps["bhp"], maps["onesp"]
    return maps


def kernel(x, Wf, bf, Wg, bg, Wh, bh, Wv, bv, gamma):
    from concourse.bass_utils import run_bass_kernel_spmd

    x = np.asarray(x, np.float32)
    b, hh, ww, c = x.shape
    n = hh * ww
    assert (b, c) == (B, C)

    hbz = bool(np.all(np.asarray(bh) == 0))
    nc = get_program(n, hbz)
    base = make_weight_maps(Wf, bf, Wg, bg, Wh, bh, Wv, bv, gamma, hbz)
    xf = x.reshape(b, n, c)
    in_maps = [dict(base, x=np.ascontiguousarray(xf[i])) for i in range(b)]

    res = run_bass_kernel_spmd(nc, in_maps, core_ids=list(range(b)))
    out = np.stack([res.results[i]["out"] for i in range(b)], axis=0)
    return np.ascontiguousarray(out.reshape(b, hh, ww, c).astype(np.float32))


# revision 39
# speedup vs baseline: 1.0869x; 1.0130x over previous
"""SAGAN self-attention block on 8 TRN2 NeuronCores.

Reference (per batch element b, N = H*W = 4096, C = 512, D = 64):
    f = x @ Wf + bf ; g = x @ Wg + bg ; h = x @ Wh + bh      # [N, D]
    s = f @ g.T                                              # [N, N]
    attn = softmax(s, axis=-1)
    ctx = attn @ h                                           # [N, D]
    o = (gamma * ctx) @ Wv + bv + x                          # [N, C]

Sharding: data-parallel over batch B=8 -> one batch element per core, no
collectives. Weights replicated.

Device algorithm (per core), matmuls in bf16 with f32 PSUM accumulation:
  - load x [4096, 512] f32; transpose 128x128 blocks on the PE (f32), cast
    to bf16 on the PSUM->SBUF copy -> xT (c on partitions)
  - f and g projected in ONE matmul chain per 512-chunk using stacked
    [Wf|Wg] weights (M=128): fT lands on PSUM partitions 0:64, gT on
    64:128; each half is then mirrored into the other partition half of
    FT2/GT2 via SBUF->SBUF DMA so QK pairs can row-pack.
  - h_aug[m, :] = [x@Wh + bh, 1.0]  -> [4096, 65] bf16 (m on partitions);
    bh is applied by a K=1 matmul against a ones row.
  - unnormalized softmax (no max subtraction: |s| <~ 50 so exp fits f32/bf16):
      for each n-chunk of 512 columns:
        for each pair (i0, i1) of 128-row m-tiles:
          S'[i0]|S'[i1] computed CONCURRENTLY via K=64 row-packing
          (tile_position (0,0) and (64,0)) into one [128, 1024] PSUM tile
          E' = exp(S')  (ScalarE, one 1024-wide call)  -> SBUF bf16
          ctxT[0:65, chunk] += haug[i0].T @ E'[:, :512] + haug[i1].T @ E'[:, 512:]
        row 64 of ctxT = sum_m E' = softmax denominator (ones column trick)
  - out[n, :] = (ctxT[:, n].T @ [gamma*Wv ; bv]) * (1/denom[n]) + x[n, :]
      (bv rides on the denom row so it survives the 1/denom scaling)
"""

import numpy as np
import ml_dtypes

BF16 = ml_dtypes.bfloat16

B, HH, WW, C = 8, 64, 64, 512
D = C // 8          # 64
N_FULL = HH * WW    # 4096
P = 128
CC = C // P         # 4  (c-chunks of 128)

_CACHE: dict = {}


def _build(n: int, h_bias_zero: bool = False):
    """Build + compile the single-core Bass program (same NEFF on all 8 cores)."""
    import concourse.mybir as mybir
    from concourse import bacc
    from concourse.tile import TileContext

    f32 = mybir.dt.float32
    bf16 = mybir.dt.bfloat16
    ADD = mybir.AluOpType.add
    MULT = mybir.AluOpType.mult
    EXP = mybir.ActivationFunctionType.Exp

    n_tiles = n // P
    n_pairs = n_tiles // 2
    nch = n // 512          # number of 512-wide n-chunks

    nc = bacc.Bacc("TRN2", target_bir_lowering=False, debug=False)

    x_d = nc.dram_tensor("x", [n, C], f32, kind="ExternalInput")
    wfg_d = nc.dram_tensor("wfg", [C, 2 * D], bf16, kind="ExternalInput")
    wh_d = nc.dram_tensor("wh", [C, D], bf16, kind="ExternalInput")
    bfg_d = nc.dram_tensor("bfg", [P, 1], f32, kind="ExternalInput")   # [bf;bg]
    if not h_bias_zero:
        bh_d = nc.dram_tensor("bhp", [1, D], bf16, kind="ExternalInput")
        on_d = nc.dram_tensor("onesp", [1, P], bf16, kind="ExternalInput")
    wv_d = nc.dram_tensor("wv", [D + 1, C], bf16, kind="ExternalInput")
    id_d = nc.dram_tensor("ident", [P, P], bf16, kind="ExternalInput")
    out_d = nc.dram_tensor("out", [n, C], f32, kind="ExternalOutput")

    x_t = x_d.rearrange("(i p) c -> i p c", p=P)
    o_t = out_d.rearrange("(i p) c -> i p c", p=P)

    with TileContext(nc) as tc:
        with (
            tc.tile_pool(name="const", bufs=1) as cpool,
            tc.tile_pool(name="big", bufs=1) as bigpool,
            tc.tile_pool(name="ep", bufs=4) as epool,
            tc.tile_pool(name="ct", bufs=3) as ctpool,
            tc.tile_pool(name="os", bufs=4) as opool,
            tc.tile_pool(name="sm", bufs=4) as smpool,
            tc.tile_pool(name="psA", bufs=2, space="PSUM") as psA,
            tc.tile_pool(name="psB", bufs=2, space="PSUM") as psB,
            tc.tile_pool(name="psC", bufs=2, space="PSUM") as psC,
        ):
            # ---- replicated constants -> SBUF
            wfg_sb = cpool.tile([P, CC, 2 * D], bf16)
            nc.sync.dma_start(wfg_sb, wfg_d.rearrange("(cc p) d -> p cc d", p=P))
            wh_sb = cpool.tile([P, CC, D], bf16)
            nc.sync.dma_start(wh_sb, wh_d.rearrange("(cc p) d -> p cc d", p=P))
            bfg_sb = cpool.tile([P, 1], f32)
            nc.sync.dma_start(bfg_sb, bfg_d[:, :])
            if not h_bias_zero:
                bh_sb = cpool.tile([1, D], bf16)
                nc.sync.dma_start(bh_sb, bh_d[:, :])
                ones_sb = cpool.tile([1, P], bf16)
                nc.sync.dma_start(ones_sb, on_d[:, :])
            wv_sb = cpool.tile([D + 1, C], bf16)
            nc.sync.dma_start(wv_sb, wv_d[:, :])
            id_sb = cpool.tile([P, P], bf16)
            nc.sync.dma_start(id_sb, id_d[:, :])
            idf_sb = cpool.tile([P, P], f32)
            nc.vector.tensor_copy(out=idf_sb, in_=id_sb)

            # ---- persistent SBUF tensors
            xres = bigpool.tile([P, n_tiles, C], f32)    # x rows (residual + cast src)
            xT = bigpool.tile([P, CC, n], bf16)          # x transposed (c on partitions)
            FT2 = bigpool.tile([P, n], bf16)             # f.T duplicated in both halves
            GT2 = bigpool.tile([P, n], bf16)             # g.T duplicated in both halves
            haug = bigpool.tile([P, n_tiles, D + 1], bf16)
            nc.gpsimd.memset(haug[:, :, D:D + 1], 1.0)

            # ---- prologue: load x, transpose (PE), project f/g/h.
            # Emission is interleaved per x-tile so scheduler priorities follow
            # the dataflow: tile i's transposes, then h(i), then the f/g chunk
            # as soon as its 4 tiles are in.
            def emit_fg(jc):
                sl = slice(jc * 512, (jc + 1) * 512)
                fg = psC.tile([P, 512], f32, tag="oc", name=f"fg{jc}")
                for cc in range(CC):
                    nc.tensor.matmul(
                        fg, lhsT=wfg_sb[:, cc, :], rhs=xT[:, cc, sl],
                        start=(cc == 0), stop=(cc == CC - 1),
                    )
                nc.vector.tensor_scalar(FT2[0:D, sl], fg[0:D, :], bfg_sb[0:D], None, ADD)
                nc.vector.tensor_scalar(GT2[D:P, sl], fg[D:P, :], bfg_sb[D:P], None, ADD)
                nc.sync.dma_start(FT2[D:P, sl], FT2[0:D, sl])
                nc.sync.dma_start(GT2[0:D, sl], GT2[D:P, sl])

            def emit_h(i):
                hps = psC.tile([P, D], f32, tag="oc", name=f"hps{i}")
                for cc in range(CC):
                    nc.tensor.matmul(
                        hps, lhsT=xT[:, cc, i * P:(i + 1) * P], rhs=wh_sb[:, cc, :],
                        start=(cc == 0), stop=(h_bias_zero and cc == CC - 1),
                    )
                if not h_bias_zero:
                    nc.tensor.matmul(
                        hps, lhsT=ones_sb, rhs=bh_sb, start=False, stop=True)
                nc.vector.tensor_copy(out=haug[:, i, 0:D], in_=hps)

            for i in range(n_tiles):
                nc.sync.dma_start(xres[:, i, :], x_t[i])
                tp = psA.tile([P, C], f32, tag="sp")
                for cc in range(CC):
                    nc.tensor.transpose(
                        tp[:, cc * P:(cc + 1) * P],
                        xres[:, i, cc * P:(cc + 1) * P], idf_sb
                    )
                nc.vector.tensor_copy(
                    out=xT[:, :, i * P:(i + 1) * P],
                    in_=tp.rearrange("p (cc q) -> p cc q", q=P),
                )
                emit_h(i)
                if i % 4 == 3:
                    emit_fg(i // 4)

            # ---- attention main loop: n-chunks of 512, m-tiles in packed pairs
            for jc in range(nch):
                sl = slice(jc * 512, (jc + 1) * 512)
                ctx = psB.tile([D + 1, 512], f32, tag="ctx")
                for ip in range(n_pairs):
                    i0, i1 = 2 * ip, 2 * ip + 1
                    sp = psA.tile([P, 1024], f32, tag="sp")
                    # two K=64 QK matmuls run concurrently in array row groups
                    nc.tensor.matmul(
                        sp[:, 0:512],
                        lhsT=GT2[0:D, i0 * P:(i0 + 1) * P], rhs=FT2[0:D, sl],
                        start=True, stop=True, tile_position=(0, 0),
                    )
                    nc.tensor.matmul(
                        sp[:, 512:1024],
                        lhsT=GT2[D:P, i1 * P:(i1 + 1) * P], rhs=FT2[D:P, sl],
                        start=True, stop=True, tile_position=(D, 0),
                    )
                    ep = epool.tile([P, 1024], bf16, tag="ep")
                    nc.scalar.activation(ep, sp, EXP)
                    nc.tensor.matmul(
                        ctx, lhsT=haug[:, i0, :], rhs=ep[:, 0:512],
                        start=(ip == 0), stop=False,
                    )
                    nc.tensor.matmul(
                        ctx, lhsT=haug[:, i1, :], rhs=ep[:, 512:1024],
                        start=False, stop=(ip == n_pairs - 1),
                    )

                # ---- epilogue for this n-chunk (4 subtiles of 128 rows)
                ct = ctpool.tile([D + 1, 512], bf16, tag="ct")
                nc.vector.tensor_copy(out=ct, in_=ctx)
                for t in range(4):
                    it = jc * 4 + t
                    tsl = slice(t * P, (t + 1) * P)
                    dt = psC.tile([P, 1], bf16, tag="oc")
                    nc.tensor.transpose(dt, ct[D:D + 1, tsl], id_sb[D:D + 1, D:D + 1])
                    rc = smpool.tile([P, 1], f32, tag="rc")
                    nc.vector.reciprocal(rc, dt)
                    op = psC.tile([P, C], f32, tag="oc")
                    nc.tensor.matmul(op, lhsT=ct[:, tsl], rhs=wv_sb, start=True, stop=True)
                    osb = opool.tile([P, C], f32, tag="os")
                    nc.vector.tensor_scalar(osb, op, rc, None, MULT)
                    nc.vector.tensor_tensor(osb, osb, xres[:, it, :], ADD)
                    if jc == nch - 1:
                        # the final chunk's stores are the kernel tail: split
                        # them across two queues to halve the drain latency
                        nc.sync.dma_start(o_t[it][0:D, :], osb[0:D, :])
                        nc.sync.dma_start(o_t[it][D:P, :], osb[D:P, :])
                    else:
                        nc.sync.dma_start(o_t[it], osb)

    nc.compile()
    return nc


def get_program(n: int = N_FULL, h_bias_zero: bool = False):
    key = (n, h_bias_zero)
    if key not in _CACHE:
        _CACHE[key] = _build(n, h_bias_zero)
    return _CACHE[key]


def make_weight_maps(Wf, bf, Wg, bg, Wh, bh, Wv, bv, gamma, h_bias_zero=False):
    """Host-side layout prep of the tiny replicated weights."""
    wv_aug = np.concatenate(
        [np.float32(gamma) * np.asarray(Wv, np.float32),
         np.asarray(bv, np.float32)[None, :]], axis=0)
    bfg = np.concatenate(
        [np.asarray(bf, np.float32), np.asarray(bg, np.float32)]).reshape(P, 1)
    wfg = np.concatenate(
        [np.asarray(Wf, np.float32), np.asarray(Wg, np.float32)], axis=1)
    maps = {
        "wfg": np.ascontiguousarray(wfg.astype(BF16)),
        "wh": np.ascontiguousarray(np.asarray(Wh, np.float32).astype(BF16)),
        "bfg": np.ascontiguousarray(bfg),
        "bhp": np.ascontiguousarray(np.asarray(bh, np.float32).astype(BF16).reshape(1, D)),
        "onesp": np.ones((1, P), dtype=BF16),
        "wv": np.ascontiguousarray(wv_aug.astype(BF16)),
        "ident": np.ascontiguousarray(np.eye(P, dtype=BF16)),
    }
    if h_bias_zero:
        del maps["bhp"], maps["onesp"]
    return maps


def kernel(x, Wf, bf, Wg, bg, Wh, bh, Wv, bv, gamma):
    from concourse.bass_utils import run_bass_kernel_spmd

    x = np.asarray(x, np.float32)
    b, hh, ww, c = x.shape
    n = hh * ww
    assert (b, c) == (B, C)

    hbz = bool(np.all(np.asarray(bh) == 0))
    nc = get_program(n, hbz)
    base = make_weight_maps(Wf, bf, Wg, bg, Wh, bh, Wv, bv, gamma, hbz)
    xf = x.reshape(b, n, c)
    in_maps = [dict(base, x=np.ascontiguousarray(xf[i])) for i in range(b)]

    res = run_bass_kernel_spmd(nc, in_maps, core_ids=list(range(b)))
    out = np.stack([res.results[i]["out"] for i in range(b)], axis=0)
    return np.ascontiguousarray(out.reshape(b, hh, ww, c).astype(np.float32))
